# revision 17
# baseline (speedup 1.0000x reference)
"""GQA self-attention block (q/k/v proj + causal softmax attention + o proj)
on 8 trn2 NeuronCores.

Sharding: batch (2) x query-head-groups (4) -> 8 cores. Core c handles
batch b=c//4 and heads [8g, 8g+8) where g=c%4 (kv heads [2g, 2g+2)).
Each core computes a partial transposed output [D, T] = o_proj_cols @ ctx;
the host sums the 4 partials per batch and transposes (all-reduce host-side).

v2 design notes (from perfetto trace of the fp32r baseline: PE stream 255us
but PE-array occupancy 437us -> ~180us of unhidden stationary loads, phase-2
at 35% stream efficiency, phases fully serial):
  - everything bf16 (same 1 cycle/row PE stream rate as fp32r, but half the
    stationary-load time, half the DMA bytes, no moving>=256 restriction so
    the causal diagonal is trimmed to 128-col granularity).
  - phase 1 streams 1024-token blocks per weight load (16-matmul PSUM
    accumulation chains hide the loads entirely).
  - phase 2 keeps keys-stationary chunks but emits the two per-chunk ctx
    matmuls as one strided-AP matmul, exp as one strided ACT op.
  - phase 3 computes out.T (o_proj cols stationary, ctx moving) so each
    stationary load streams 1024 tokens; host transposes.
  - phases interleaved for engine overlap: phase-2 rounds for query blocks
    a=0,1 run between phase-1 groups (ACT exp overlaps PE projections);
    phase-1 second half runs inside the a=0,1 round stream; phase-3 first
    token half is interleaved into the a=2,3 rounds (fills PE while ACT
    does exp); only the second token half of phase 3 trails.
  - v transposed via XBAR dma_start_transpose (no PE/PSUM involvement).
"""

import os
import sys

sys.path.insert(0, "/opt/trn_rl_repo")

import numpy as np

import concourse.bass as bass
import concourse.tile as tile
from concourse import bacc, mybir
from concourse.bass_utils import run_bass_kernel_spmd

F32 = mybir.dt.float32
BF16 = mybir.dt.bfloat16
EXP = mybir.ActivationFunctionType.Exp

B, T, D = 2, 2048, 2048
HQ, HK = 32, 8
DH = D // HQ              # 64 head dim
N_CORES = 8
GROUPS = 4                # head groups per batch
QCOLS = D // GROUPS       # 512 q cols per core
KCOLS = (D // 4) // GROUPS  # 128 k cols per core (2 kv heads)
WCOLS = QCOLS + 2 * KCOLS   # 768
KT = D // 128             # 16 contraction tiles
TB = 1024                 # phase-1/3 token block (half of T)
NEG = -480.0              # additive mask pre-scale (-60 after 1/8 scale)

_cache = {}


def _build():
    nc = bacc.Bacc("TRN2", target_bir_lowering=False, debug=False)

    xT_d = nc.declare_dram_parameter("xT", [D, T], BF16, isOutput=False)
    wqkv_d = nc.declare_dram_parameter("wqkv", [D, WCOLS], BF16, isOutput=False)
    oproj_d = nc.declare_dram_parameter("oproj", [128, 4, D], BF16, isOutput=False)
    masks_d = nc.declare_dram_parameter("masks", [128, 2, 128], BF16, isOutput=False)
    ones_d = nc.declare_dram_parameter("ones", [128, 16 * 80], BF16, isOutput=False)
    out_d = nc.declare_dram_parameter("out", [D, T], BF16, isOutput=True)
    rcscr_d = nc.dram_tensor("rcscratch", [16, 1024], F32)

    with tile.TileContext(nc) as tc:
        with (
            tc.tile_pool(name="pers", bufs=1) as pers,
            tc.tile_pool(name="xt", bufs=20) as xtp,
            tc.tile_pool(name="ep", bufs=4) as epool,
            tc.tile_pool(name="work", bufs=2) as work,
            tc.tile_pool(name="psum", bufs=1, space="PSUM") as psum,
        ):
            # ---- persistent SBUF ----
            wqkv_sb = pers.tile([128, KT, WCOLS], BF16, tag="wqkv")
            oproj_sb = pers.tile([128, 4, D], BF16, tag="oproj")
            masks_sb = pers.tile([128, 2, 128], BF16, tag="masks")
            qt = [pers.tile([128, T], BF16, tag=f"qt{m}", name=f"qt{m}")
                  for m in range(4)]
            kp = [pers.tile([128, T], BF16, tag=f"kp{k}", name=f"kp{k}")
                  for k in range(2)]
            vT = pers.tile([128, T], BF16, tag="vT")
            vs = [pers.tile([128, 16 * 80], BF16, tag=f"vs{k}", name=f"vs{k}")
                  for k in range(2)]
            ctx = [pers.tile([128, T], BF16, tag=f"ctx{m}", name=f"ctx{m}")
                   for m in range(4)]

            # ---- weight/constant DMAs (w[k] just ahead of x[k]) ----
            wq_r = wqkv_d[:].rearrange("(k p) c -> p k c", p=128)
            xts = {}

            def load_x_half(half, with_w=False):
                ts = slice(TB * half, TB * half + TB)
                tiles = []
                for k in range(KT):
                    xt = xtp.tile([128, TB], BF16, tag=f"xt{half}", bufs=16,
                                  name="xt")
                    nc.sync.dma_start(xt, xT_d[128 * k: 128 * k + 128, ts])
                    tiles.append(xt)
                    if with_w:
                        nc.sync.dma_start(wqkv_sb[:, k, :], wq_r[:, k, :])
                    if with_w and k == 3:
                        nc.sync.dma_start(masks_sb, masks_d[:])
                        nc.sync.dma_start(vs[0], ones_d[:])
                        nc.sync.dma_start(vs[1], ones_d[:])
                xts[half] = tiles

            def v2(ap):
                return ap.rearrange("p (h q) -> p h q", h=2)

            # ---- phase 1: one 16-matmul chain per (m, token-half) ----
            def p1_group(mi, half, eng):
                ts = slice(TB * half, TB * half + TB)
                ps = psum.tile([128, TB], F32, tag="s2", bufs=3, name="p1ps")
                # matmul out must stay within one PSUM bank (512 f32 cols)
                for sub in (0, 512):
                    for k in range(KT):
                        nc.tensor.matmul(
                            ps[:, sub: sub + 512],
                            wqkv_sb[:, k, 128 * mi: 128 * mi + 128],
                            xts[half][k][:, sub: sub + 512],
                            start=(k == 0),
                            stop=(k == KT - 1),
                        )
                copy = eng.copy if eng is nc.scalar else eng.tensor_copy
                if mi < 4:
                    copy(qt[mi][:, ts], ps)
                elif mi == 4:
                    copy(kp[0][0:64, ts], ps[0:64, :])
                    copy(kp[1][64:128, ts], ps[64:128, :])
                    # kT duplicates at the other partition half
                    nc.sync.dma_start(kp[0][64:128, ts], kp[0][0:64, ts])
                    nc.sync.dma_start(kp[1][0:64, ts], kp[1][64:128, ts])
                else:
                    copy(vT[:, ts], ps)
                    # v transposed into vs via one XBAR dma per kv head
                    # (3D out: key j lands at partition j%128, chunk j//128;
                    # ones at col 64 of each 80-block -> softmax denominator).
                    # Issued on the ACT queue: these are ~us-scale serial ops
                    # that would head-block every dma dispatch on Sync.
                    for kv in range(2):
                        vsr = vs[kv][:].rearrange("p (c e) -> p c e", e=80)
                        nc.sync.dma_start_transpose(
                            vsr[:, 8 * half: 8 * half + 8, 0:64],
                            vT[64 * kv: 64 * kv + 64, ts],
                        )

            # ---- phase 2: attention round per (head pair m, 512-query a) ----
            pending_norm = []

            def flush_norm(keep=0):
                while len(pending_norm) > keep:
                    pending_norm.pop(0)()

            def p2_S(m, a, defer_all=False):
                """S/exp part of an attention round. With defer_all, no ctx
                matmuls are emitted (caller interposes independent PE work
                and calls p2_fin); otherwise drains run 2 chunks behind."""
                kv = m // 2
                nj = 4 * (a + 1)
                qb = 512 * a
                state = {"m": m, "a": a, "kv": kv, "pend": []}
                pend = state["pend"]

                def drain(last):
                    pE, pjc, plo = pend.pop(0)
                    for h2 in range(2):
                        nc.tensor.matmul(
                            state["ctxAB"][:, 512 * h2 + plo: 512 * h2 + 512],
                            vs[kv][:, 80 * pjc: 80 * pjc + 65],
                            pE[:, 512 * h2 + plo: 512 * h2 + 512],
                            start=(pjc == 0),
                            stop=last,
                        )
                state["drain"] = drain

                if not defer_all:
                    state["ctxAB"] = psum.tile([65, 1024], F32, tag="ctx",
                                               bufs=1, name="ctxAB")
                for jc in range(nj):
                    o = jc - 4 * a
                    lo = (0, 128, 256, 384)[o] if o >= 0 else 0
                    if jc == 2:
                        flush_norm(1)
                    S = psum.tile([128, 1024], F32, tag="s2", bufs=3, name="S")
                    for h2 in range(2):
                        nc.tensor.matmul(
                            S[:, 512 * h2 + lo: 512 * h2 + 512],
                            kp[kv][64 * h2: 64 * h2 + 64,
                                   128 * jc: 128 * jc + 128],
                            qt[m][64 * h2: 64 * h2 + 64, qb + lo: qb + 512],
                            start=True,
                            stop=True,
                            tile_position=(64 * h2, 0),
                        )
                    if o >= 0:
                        nc.vector.tensor_add(
                            v2(S)[:, :, lo: lo + 128],
                            v2(S)[:, :, lo: lo + 128],
                            masks_sb,
                        )
                    E = epool.tile([128, 1024], BF16, tag="E", bufs=8, name="E")
                    if lo == 0:
                        nc.scalar.activation(E, S, EXP, scale=0.125)
                    else:
                        nc.scalar.activation(
                            v2(E)[:, :, lo:512], v2(S)[:, :, lo:512],
                            EXP, scale=0.125,
                        )
                    pend.append((E, jc, lo))
                    if not defer_all and len(pend) > 2:
                        drain(False)
                return state

            def p2_fin(state):
                m, a, pend = state["m"], state["a"], state["pend"]
                drain = state["drain"]
                if "ctxAB" not in state:
                    state["ctxAB"] = psum.tile([65, 1024], F32, tag="ctx",
                                               bufs=1, name="ctxAB")
                ctxAB = state["ctxAB"]
                while pend:
                    drain(len(pend) == 1)

                # denominator path starts immediately (DMA/rcp only);
                # the DVE multiplies are deferred one round so they never
                # stall this round's pipeline waiting on the DRAM bounce.
                cu = work.tile([65, 1024], F32, tag="cu", bufs=3, name="cu")
                nc.vector.tensor_copy(cu, ctxAB)
                den128 = work.tile([128, 8], F32, tag="d128", bufs=2,
                                   name="den128")
                nc.sync.dma_start(den128, cu[64:65, :])
                rcp = work.tile([128, 8], F32, tag="rcp", bufs=2, name="rcp")
                nc.vector.reciprocal(rcp, den128)
                ma = m * 4 + a
                nc.sync.dma_start(rcscr_d[ma: ma + 1, :], rcp)
                bcs = work.tile([64, 1024], F32, tag="bcs", bufs=3,
                                name="bcs")
                nc.sync.dma_start(
                    bcs, rcscr_d[ma: ma + 1, :].partition_broadcast(64)
                )

                def _norm(cu=cu, bcs=bcs, m=m, a=a):
                    isl = slice(512 * a, 512 * a + 512)
                    nc.vector.tensor_mul(
                        ctx[m][0:64, isl], cu[0:64, 0:512], bcs[:, 0:512]
                    )
                    tmpB = work.tile([64, 512], BF16, tag="tb", bufs=2,
                                     name="tmpB")
                    nc.vector.tensor_mul(
                        tmpB, cu[0:64, 512:1024], bcs[:, 512:1024]
                    )
                    nc.sync.dma_start(ctx[m][64:128, isl], tmpB)

                pending_norm.append(_norm)

            def p2_round(m, a):
                p2_fin(p2_S(m, a))

            # ---- phase 3: out.T group per (128-outcol chunk, token half) ----
            def p3_group(rc, th, eng=None):
                eng = eng or nc.vector
                tsl = slice(TB * th, TB * th + TB)
                ps3 = psum.tile([128, TB], F32, tag="s2", bufs=3, name="p3ps")
                for sub in (0, 512):
                    for m in range(4):
                        nc.tensor.matmul(
                            ps3[:, sub: sub + 512],
                            oproj_sb[:, m, 128 * rc: 128 * rc + 128],
                            ctx[m][:, TB * th + sub: TB * th + sub + 512],
                            start=(m == 0),
                            stop=(m == 3),
                        )
                ostage = work.tile([128, TB], BF16, tag="ostage", bufs=3,
                                   name="ostage")
                if eng is nc.scalar:
                    eng.copy(ostage, ps3)
                else:
                    eng.tensor_copy(ostage, ps3)
                nc.sync.dma_start(out_d[128 * rc: 128 * rc + 128, tsl], ostage)

            # ================= emission =================
            # stage 1: a=0 rounds cross-pipelined with phase-1 chains (the
            # 4-chunk rounds are too short to hide their own exp latency)
            load_x_half(0, with_w=True)
            p1_group(4, 0, nc.scalar)
            load_x_half(1)
            p1_group(5, 0, nc.scalar)
            p1_group(0, 0, nc.vector)
            st = p2_S(0, 0, defer_all=True)
            p1_group(1, 0, nc.scalar)
            p2_fin(st)
            st = p2_S(1, 0, defer_all=True)
            p1_group(2, 0, nc.vector)
            p2_fin(st)
            st = p2_S(2, 0, defer_all=True)
            p1_group(3, 0, nc.scalar)
            p2_fin(st)
            st = p2_S(3, 0, defer_all=True)

            nc.sync.dma_start(oproj_sb, oproj_d[:])
            p1_group(4, 1, nc.vector)
            p2_fin(st)
            st = p2_S(0, 1, defer_all=True)
            p1_group(5, 1, nc.vector)
            p2_fin(st)
            p2_round(1, 1)
            p1_group(0, 1, nc.vector)
            p2_round(2, 1)
            p1_group(1, 1, nc.vector)
            p2_round(3, 1)
            p1_group(2, 1, nc.vector)
            p1_group(3, 1, nc.vector)

            flush_norm(0)
            p3c = 0
            for a in (2, 3):
                for m in range(4):
                    p2_round(m, a)
                    if p3c < 12:
                        p3_group(p3c, 0)
                        p3c += 1
                    if p3c < 12 and not (a == 3 and m >= 2):
                        p3_group(p3c, 0)
                        p3c += 1
            # remaining token-half-0 groups keep PE fed while the last
            # rounds' normalize chains (DVE + DRAM bounce) complete
            flush_norm(0)
            for rc in range(p3c, 16):
                p3_group(rc, 0)
            for rc in range(16):
                p3_group(rc, 1, nc.scalar if rc % 2 == 0 else nc.vector)

    nc.compile()
    return nc


def _host_inputs(x, q_proj, k_proj, v_proj, o_proj):
    """Per-core input dicts (numpy, bf16)."""
    import ml_dtypes
    bf = ml_dtypes.bfloat16

    jj = np.arange(128)[:, None]
    cc = np.arange(128)[None, :]
    tri = np.where(jj <= cc, 0.0, NEG).astype(np.float32)
    masks = np.stack([tri, tri], axis=1).astype(bf)  # [128, 2, 128]
    ones = np.ones((128, 16 * 80), dtype=np.float32).astype(bf)

    xT = [np.ascontiguousarray(x[b].T).astype(bf) for b in range(B)]
    in_maps = []
    for c in range(N_CORES):
        b, g = divmod(c, GROUPS)
        wqkv = np.concatenate(
            [
                q_proj[QCOLS * g: QCOLS * g + QCOLS].T,
                k_proj[KCOLS * g: KCOLS * g + KCOLS].T,
                v_proj[KCOLS * g: KCOLS * g + KCOLS].T,
            ],
            axis=1,
        ).astype(bf)
        op = o_proj[:, QCOLS * g: QCOLS * g + QCOLS].T  # [512 e, 2048 r]
        op = np.ascontiguousarray(
            op.reshape(4, 128, D).transpose(1, 0, 2)
        ).astype(bf)
        in_maps.append(
            {
                "xT": xT[b],
                "wqkv": np.ascontiguousarray(wqkv),
                "oproj": op,
                "masks": masks,
                "ones": ones,
            }
        )
    return in_maps


def run(x, q_proj, k_proj, v_proj, o_proj, trace=False):
    """Run on hardware; returns (output [B,T,D] f32, BassKernelResults)."""
    if "nc" not in _cache:
        _cache["nc"] = _build()
    nc = _cache["nc"]
    in_maps = _host_inputs(x, q_proj, k_proj, v_proj, o_proj)
    res = run_bass_kernel_spmd(
        nc, in_maps, core_ids=list(range(N_CORES)), trace=trace
    )
    parts = [res.results[c]["out"] for c in range(N_CORES)]
    out = np.empty((B, T, D), dtype=np.float32)
    for b in range(B):
        acc = parts[4 * b].astype(np.float64)
        for g in range(1, GROUPS):
            acc += parts[4 * b + g].astype(np.float64)
        out[b] = acc.T.astype(np.float32)
    return out, res


def kernel(x, q_proj, k_proj, v_proj, o_proj, hq=None, hk=None, **_unused):
    x = np.asarray(x, dtype=np.float32)
    q_proj = np.asarray(q_proj, dtype=np.float32)
    k_proj = np.asarray(k_proj, dtype=np.float32)
    v_proj = np.asarray(v_proj, dtype=np.float32)
    o_proj = np.asarray(o_proj, dtype=np.float32)
    assert x.shape == (B, T, D), x.shape
    trace = bool(os.environ.get("KERNEL_TRACE"))
    out, _ = run(x, q_proj, k_proj, v_proj, o_proj, trace=trace)
    return out


# revision 18
# speedup vs baseline: 1.1662x; 1.1662x over previous
"""GQA self-attention block (q/k/v proj + causal softmax attention + o proj)
on 8 trn2 NeuronCores.

Sharding: batch (2) x query-head-groups (4) -> 8 cores. Core c handles
batch b=c//4 and heads [8g, 8g+8) where g=c%4 (kv heads [2g, 2g+2)).
Each core computes a partial transposed output [D, T] = o_proj_cols @ ctx;
the host sums the 4 partials per batch and transposes (all-reduce host-side).

v2 design notes (from perfetto trace of the fp32r baseline: PE stream 255us
but PE-array occupancy 437us -> ~180us of unhidden stationary loads, phase-2
at 35% stream efficiency, phases fully serial):
  - everything bf16 (same 1 cycle/row PE stream rate as fp32r, but half the
    stationary-load time, half the DMA bytes, no moving>=256 restriction so
    the causal diagonal is trimmed to 128-col granularity).
  - phase 1 streams 1024-token blocks per weight load (16-matmul PSUM
    accumulation chains hide the loads entirely).
  - phase 2 keeps keys-stationary chunks but emits the two per-chunk ctx
    matmuls as one strided-AP matmul, exp as one strided ACT op.
  - phase 3 computes out.T (o_proj cols stationary, ctx moving) so each
    stationary load streams 1024 tokens; host transposes.
  - phases interleaved for engine overlap: phase-2 rounds for query blocks
    a=0,1 run between phase-1 groups (ACT exp overlaps PE projections);
    phase-1 second half runs inside the a=0,1 round stream; phase-3 first
    token half is interleaved into the a=2,3 rounds (fills PE while ACT
    does exp); only the second token half of phase 3 trails.
  - v transposed via XBAR dma_start_transpose (no PE/PSUM involvement).
"""

import os
import sys

sys.path.insert(0, "/opt/trn_rl_repo")

import numpy as np

import concourse.bass as bass
import concourse.tile as tile
from concourse import bacc, mybir
from concourse.bass_utils import run_bass_kernel_spmd

F32 = mybir.dt.float32
BF16 = mybir.dt.bfloat16
EXP = mybir.ActivationFunctionType.Exp

B, T, D = 2, 2048, 2048
HQ, HK = 32, 8
DH = D // HQ              # 64 head dim
N_CORES = 8
GROUPS = 4                # head groups per batch
QCOLS = D // GROUPS       # 512 q cols per core
KCOLS = (D // 4) // GROUPS  # 128 k cols per core (2 kv heads)
WCOLS = QCOLS + 2 * KCOLS   # 768
KT = D // 128             # 16 contraction tiles
TB = 1024                 # phase-1/3 token block (half of T)
NEG = -480.0              # additive mask pre-scale (-60 after 1/8 scale)

_cache = {}


def _build():
    nc = bacc.Bacc("TRN2", target_bir_lowering=False, debug=False)

    xT_d = nc.declare_dram_parameter("xT", [D, T], BF16, isOutput=False)
    wqkv_d = nc.declare_dram_parameter("wqkv", [D, WCOLS], BF16, isOutput=False)
    oproj_d = nc.declare_dram_parameter("oproj", [128, 4, D], BF16, isOutput=False)
    masks_d = nc.declare_dram_parameter("masks", [128, 2, 128], BF16, isOutput=False)
    ones_d = nc.declare_dram_parameter("ones", [128, 16 * 80], BF16, isOutput=False)
    out_d = nc.declare_dram_parameter("out", [D, T], BF16, isOutput=True)
    rcscr_d = nc.dram_tensor("rcscratch", [16, 1024], F32)

    with tile.TileContext(nc) as tc:
        with (
            tc.tile_pool(name="pers", bufs=1) as pers,
            tc.tile_pool(name="xt", bufs=20) as xtp,
            tc.tile_pool(name="ep", bufs=4) as epool,
            tc.tile_pool(name="work", bufs=2) as work,
            tc.tile_pool(name="psum", bufs=1, space="PSUM") as psum,
        ):
            # ---- persistent SBUF ----
            wqkv_sb = pers.tile([128, KT, WCOLS], BF16, tag="wqkv")
            oproj_sb = pers.tile([128, 4, D], BF16, tag="oproj")
            masks_sb = pers.tile([128, 2, 128], BF16, tag="masks")
            qt = [pers.tile([128, T], BF16, tag=f"qt{m}", name=f"qt{m}")
                  for m in range(4)]
            kp = [pers.tile([128, T], BF16, tag=f"kp{k}", name=f"kp{k}")
                  for k in range(2)]
            vT = pers.tile([128, T], BF16, tag="vT")
            vs = [pers.tile([128, 16 * 80], BF16, tag=f"vs{k}", name=f"vs{k}")
                  for k in range(2)]
            ctx = [pers.tile([128, T], BF16, tag=f"ctx{m}", name=f"ctx{m}")
                   for m in range(4)]

            # ---- weight/constant DMAs (w[k] just ahead of x[k]) ----
            wq_r = wqkv_d[:].rearrange("(k p) c -> p k c", p=128)
            xts = {}

            def load_x_half(half, with_w=False):
                ts = slice(TB * half, TB * half + TB)
                tiles = []
                for k in range(KT):
                    if with_w:
                        nc.sync.dma_start(wqkv_sb[:, k, :], wq_r[:, k, :])
                    xt = xtp.tile([128, TB], BF16, tag=f"xt{half}", bufs=16,
                                  name="xt")
                    nc.sync.dma_start(xt, xT_d[128 * k: 128 * k + 128, ts])
                    tiles.append(xt)
                    if with_w and k == 3:
                        nc.sync.dma_start(masks_sb, masks_d[:])
                        nc.sync.dma_start(vs[0], ones_d[:])
                        nc.sync.dma_start(vs[1], ones_d[:])
                xts[half] = tiles

            def v2(ap):
                return ap.rearrange("p (h q) -> p h q", h=2)

            # ---- phase 1: one 16-matmul chain per (m, token-half) ----
            def p1_group(mi, half, eng):
                ts = slice(TB * half, TB * half + TB)
                ps = psum.tile([128, TB], F32, tag="s2", bufs=3, name="p1ps")
                # matmul out must stay within one PSUM bank (512 f32 cols)
                for sub in (0, 512):
                    for k in range(KT):
                        nc.tensor.matmul(
                            ps[:, sub: sub + 512],
                            wqkv_sb[:, k, 128 * mi: 128 * mi + 128],
                            xts[half][k][:, sub: sub + 512],
                            start=(k == 0),
                            stop=(k == KT - 1),
                        )
                copy = eng.copy if eng is nc.scalar else eng.tensor_copy
                if mi < 4:
                    copy(qt[mi][:, ts], ps)
                elif mi == 4:
                    copy(kp[0][0:64, ts], ps[0:64, :])
                    copy(kp[1][64:128, ts], ps[64:128, :])
                    # kT duplicates at the other partition half
                    nc.sync.dma_start(kp[0][64:128, ts], kp[0][0:64, ts])
                    nc.sync.dma_start(kp[1][0:64, ts], kp[1][64:128, ts])
                else:
                    copy(vT[:, ts], ps)
                    # v transposed into vs via one XBAR dma per kv head
                    # (3D out: key j lands at partition j%128, chunk j//128;
                    # ones at col 64 of each 80-block -> softmax denominator).
                    # Issued on the ACT queue: these are ~us-scale serial ops
                    # that would head-block every dma dispatch on Sync.
                    for kv in range(2):
                        vsr = vs[kv][:].rearrange("p (c e) -> p c e", e=80)
                        nc.sync.dma_start_transpose(
                            vsr[:, 8 * half: 8 * half + 8, 0:64],
                            vT[64 * kv: 64 * kv + 64, ts],
                        )

            # ---- phase 2: attention round per (head pair m, 512-query a) ----
            pending_norm = []

            def flush_norm(keep=0):
                while len(pending_norm) > keep:
                    pending_norm.pop(0)()

            def p2_S(m, a, defer_all=False):
                """S/exp part of an attention round. With defer_all, no ctx
                matmuls are emitted (caller interposes independent PE work
                and calls p2_fin); otherwise drains run 2 chunks behind."""
                kv = m // 2
                nj = 4 * (a + 1)
                qb = 512 * a
                state = {"m": m, "a": a, "kv": kv, "pend": []}
                pend = state["pend"]

                def drain(last):
                    pE, pjc, plo = pend.pop(0)
                    for h2 in range(2):
                        nc.tensor.matmul(
                            state["ctxAB"][:, 512 * h2 + plo: 512 * h2 + 512],
                            vs[kv][:, 80 * pjc: 80 * pjc + 65],
                            pE[:, 512 * h2 + plo: 512 * h2 + 512],
                            start=(pjc == 0),
                            stop=last,
                        )
                state["drain"] = drain

                if not defer_all:
                    state["ctxAB"] = psum.tile([65, 1024], F32, tag="ctx",
                                               bufs=1, name="ctxAB")
                for jc in range(nj):
                    o = jc - 4 * a
                    lo = (0, 128, 256, 384)[o] if o >= 0 else 0
                    if jc == 2:
                        flush_norm(1)
                    S = psum.tile([128, 1024], F32, tag="s2", bufs=3, name="S")
                    for h2 in range(2):
                        nc.tensor.matmul(
                            S[:, 512 * h2 + lo: 512 * h2 + 512],
                            kp[kv][64 * h2: 64 * h2 + 64,
                                   128 * jc: 128 * jc + 128],
                            qt[m][64 * h2: 64 * h2 + 64, qb + lo: qb + 512],
                            start=True,
                            stop=True,
                            tile_position=(64 * h2, 0),
                        )
                    if o >= 0:
                        nc.vector.tensor_add(
                            v2(S)[:, :, lo: lo + 128],
                            v2(S)[:, :, lo: lo + 128],
                            masks_sb,
                        )
                    E = epool.tile([128, 1024], BF16, tag="E", bufs=8, name="E")
                    if lo == 0:
                        nc.scalar.activation(E, S, EXP, scale=0.125)
                    else:
                        nc.scalar.activation(
                            v2(E)[:, :, lo:512], v2(S)[:, :, lo:512],
                            EXP, scale=0.125,
                        )
                    pend.append((E, jc, lo))
                    if not defer_all and len(pend) > 2:
                        drain(False)
                return state

            def p2_fin(state):
                m, a, pend = state["m"], state["a"], state["pend"]
                drain = state["drain"]
                if "ctxAB" not in state:
                    state["ctxAB"] = psum.tile([65, 1024], F32, tag="ctx",
                                               bufs=1, name="ctxAB")
                ctxAB = state["ctxAB"]
                while pend:
                    drain(len(pend) == 1)

                # denominator path starts immediately (DMA/rcp only);
                # the DVE multiplies are deferred one round so they never
                # stall this round's pipeline waiting on the DRAM bounce.
                cu = work.tile([65, 1024], F32, tag="cu", bufs=3, name="cu")
                nc.vector.tensor_copy(cu, ctxAB)
                den128 = work.tile([128, 8], F32, tag="d128", bufs=2,
                                   name="den128")
                nc.sync.dma_start(den128, cu[64:65, :])
                rcp = work.tile([128, 8], F32, tag="rcp", bufs=2, name="rcp")
                nc.vector.reciprocal(rcp, den128)
                ma = m * 4 + a
                nc.sync.dma_start(rcscr_d[ma: ma + 1, :], rcp)
                bcs = work.tile([64, 1024], F32, tag="bcs", bufs=3,
                                name="bcs")
                nc.sync.dma_start(
                    bcs, rcscr_d[ma: ma + 1, :].partition_broadcast(64)
                )

                def _norm(cu=cu, bcs=bcs, m=m, a=a):
                    isl = slice(512 * a, 512 * a + 512)
                    nc.vector.tensor_mul(
                        ctx[m][0:64, isl], cu[0:64, 0:512], bcs[:, 0:512]
                    )
                    tmpB = work.tile([64, 512], BF16, tag="tb", bufs=2,
                                     name="tmpB")
                    nc.vector.tensor_mul(
                        tmpB, cu[0:64, 512:1024], bcs[:, 512:1024]
                    )
                    nc.sync.dma_start(ctx[m][64:128, isl], tmpB)

                pending_norm.append(_norm)

            def p2_round(m, a):
                p2_fin(p2_S(m, a))

            # ---- phase 3: out.T group per (128-outcol chunk, token half) ----
            def p3_group(rc, th, eng=None):
                eng = eng or nc.vector
                tsl = slice(TB * th, TB * th + TB)
                ps3 = psum.tile([128, TB], F32, tag="s2", bufs=3, name="p3ps")
                for sub in (0, 512):
                    for m in range(4):
                        nc.tensor.matmul(
                            ps3[:, sub: sub + 512],
                            oproj_sb[:, m, 128 * rc: 128 * rc + 128],
                            ctx[m][:, TB * th + sub: TB * th + sub + 512],
                            start=(m == 0),
                            stop=(m == 3),
                        )
                ostage = work.tile([128, TB], BF16, tag="ostage", bufs=3,
                                   name="ostage")
                if eng is nc.scalar:
                    eng.copy(ostage, ps3)
                else:
                    eng.tensor_copy(ostage, ps3)
                nc.sync.dma_start(out_d[128 * rc: 128 * rc + 128, tsl], ostage)

            # ================= emission =================
            # stage 1: a=0 rounds cross-pipelined with phase-1 chains (the
            # 4-chunk rounds are too short to hide their own exp latency)
            load_x_half(0, with_w=True)
            p1_group(4, 0, nc.scalar)
            load_x_half(1)
            p1_group(5, 0, nc.scalar)
            p1_group(0, 0, nc.vector)
            st = p2_S(0, 0, defer_all=True)
            p1_group(1, 0, nc.scalar)
            p2_fin(st)
            st = p2_S(1, 0, defer_all=True)
            p1_group(2, 0, nc.vector)
            p2_fin(st)
            st = p2_S(2, 0, defer_all=True)
            p1_group(3, 0, nc.scalar)
            p2_fin(st)
            st = p2_S(3, 0, defer_all=True)

            nc.sync.dma_start(oproj_sb, oproj_d[:])
            p1_group(4, 1, nc.vector)
            p2_fin(st)
            st = p2_S(0, 1, defer_all=True)
            p1_group(5, 1, nc.vector)
            p2_fin(st)
            p2_round(1, 1)
            p1_group(0, 1, nc.vector)
            p2_round(2, 1)
            p1_group(1, 1, nc.vector)
            p2_round(3, 1)
            p1_group(2, 1, nc.vector)
            p1_group(3, 1, nc.vector)

            flush_norm(0)
            p3c = 0
            for a in (2, 3):
                for m in range(4):
                    p2_round(m, a)
                    if p3c < 12:
                        p3_group(p3c, 0)
                        p3c += 1
                    if p3c < 12 and not (a == 3 and m >= 2):
                        p3_group(p3c, 0)
                        p3c += 1
            # remaining token-half-0 groups keep PE fed while the last
            # rounds' normalize chains (DVE + DRAM bounce) complete
            flush_norm(0)
            for rc in range(p3c, 16):
                p3_group(rc, 0)
            for rc in range(16):
                p3_group(rc, 1, nc.scalar if rc % 2 == 0 else nc.vector)

    nc.compile()
    return nc


def _host_inputs(x, q_proj, k_proj, v_proj, o_proj):
    """Per-core input dicts (numpy, bf16)."""
    import ml_dtypes
    bf = ml_dtypes.bfloat16

    jj = np.arange(128)[:, None]
    cc = np.arange(128)[None, :]
    tri = np.where(jj <= cc, 0.0, NEG).astype(np.float32)
    masks = np.stack([tri, tri], axis=1).astype(bf)  # [128, 2, 128]
    ones = np.ones((128, 16 * 80), dtype=np.float32).astype(bf)

    xT = [np.ascontiguousarray(x[b].T).astype(bf) for b in range(B)]
    in_maps = []
    for c in range(N_CORES):
        b, g = divmod(c, GROUPS)
        wqkv = np.concatenate(
            [
                q_proj[QCOLS * g: QCOLS * g + QCOLS].T,
                k_proj[KCOLS * g: KCOLS * g + KCOLS].T,
                v_proj[KCOLS * g: KCOLS * g + KCOLS].T,
            ],
            axis=1,
        ).astype(bf)
        op = o_proj[:, QCOLS * g: QCOLS * g + QCOLS].T  # [512 e, 2048 r]
        op = np.ascontiguousarray(
            op.reshape(4, 128, D).transpose(1, 0, 2)
        ).astype(bf)
        in_maps.append(
            {
                "xT": xT[b],
                "wqkv": np.ascontiguousarray(wqkv),
                "oproj": op,
                "masks": masks,
                "ones": ones,
            }
        )
    return in_maps


def run(x, q_proj, k_proj, v_proj, o_proj, trace=False):
    """Run on hardware; returns (output [B,T,D] f32, BassKernelResults)."""
    if "nc" not in _cache:
        _cache["nc"] = _build()
    nc = _cache["nc"]
    in_maps = _host_inputs(x, q_proj, k_proj, v_proj, o_proj)
    res = run_bass_kernel_spmd(
        nc, in_maps, core_ids=list(range(N_CORES)), trace=trace
    )
    parts = [res.results[c]["out"] for c in range(N_CORES)]
    out = np.empty((B, T, D), dtype=np.float32)
    for b in range(B):
        acc = parts[4 * b].astype(np.float64)
        for g in range(1, GROUPS):
            acc += parts[4 * b + g].astype(np.float64)
        out[b] = acc.T.astype(np.float32)
    return out, res


def kernel(x, q_proj, k_proj, v_proj, o_proj, hq=None, hk=None, **_unused):
    x = np.asarray(x, dtype=np.float32)
    q_proj = np.asarray(q_proj, dtype=np.float32)
    k_proj = np.asarray(k_proj, dtype=np.float32)
    v_proj = np.asarray(v_proj, dtype=np.float32)
    o_proj = np.asarray(o_proj, dtype=np.float32)
    assert x.shape == (B, T, D), x.shape
    trace = bool(os.environ.get("KERNEL_TRACE"))
    out, _ = run(x, q_proj, k_proj, v_proj, o_proj, trace=trace)
    return out


# revision 19
# speedup vs baseline: 1.1903x; 1.0207x over previous
"""GQA self-attention block (q/k/v proj + causal softmax attention + o proj)
on 8 trn2 NeuronCores.

Sharding: batch (2) x query-head-groups (4) -> 8 cores. Core c handles
batch b=c//4 and heads [8g, 8g+8) where g=c%4 (kv heads [2g, 2g+2)).
Each core computes a partial transposed output [D, T] = o_proj_cols @ ctx;
the host sums the 4 partials per batch and transposes (all-reduce host-side).

v2 design notes (from perfetto trace of the fp32r baseline: PE stream 255us
but PE-array occupancy 437us -> ~180us of unhidden stationary loads, phase-2
at 35% stream efficiency, phases fully serial):
  - everything bf16 (same 1 cycle/row PE stream rate as fp32r, but half the
    stationary-load time, half the DMA bytes, no moving>=256 restriction so
    the causal diagonal is trimmed to 128-col granularity).
  - phase 1 streams 1024-token blocks per weight load (16-matmul PSUM
    accumulation chains hide the loads entirely).
  - phase 2 keeps keys-stationary chunks but emits the two per-chunk ctx
    matmuls as one strided-AP matmul, exp as one strided ACT op.
  - phase 3 computes out.T (o_proj cols stationary, ctx moving) so each
    stationary load streams 1024 tokens; host transposes.
  - phases interleaved for engine overlap: phase-2 rounds for query blocks
    a=0,1 run between phase-1 groups (ACT exp overlaps PE projections);
    phase-1 second half runs inside the a=0,1 round stream; phase-3 first
    token half is interleaved into the a=2,3 rounds (fills PE while ACT
    does exp); only the second token half of phase 3 trails.
  - v transposed via XBAR dma_start_transpose (no PE/PSUM involvement).
"""

import os
import sys

sys.path.insert(0, "/opt/trn_rl_repo")

import numpy as np

import concourse.bass as bass
import concourse.tile as tile
from concourse import bacc, mybir
from concourse.bass_utils import run_bass_kernel_spmd

F32 = mybir.dt.float32
BF16 = mybir.dt.bfloat16
EXP = mybir.ActivationFunctionType.Exp

B, T, D = 2, 2048, 2048
HQ, HK = 32, 8
DH = D // HQ              # 64 head dim
N_CORES = 8
GROUPS = 4                # head groups per batch
QCOLS = D // GROUPS       # 512 q cols per core
KCOLS = (D // 4) // GROUPS  # 128 k cols per core (2 kv heads)
WCOLS = QCOLS + 2 * KCOLS   # 768
KT = D // 128             # 16 contraction tiles
TB = 1024                 # phase-1/3 token block (half of T)
NEG = -480.0              # additive mask pre-scale (-60 after 1/8 scale)

_cache = {}


def _build():
    nc = bacc.Bacc("TRN2", target_bir_lowering=False, debug=False)

    xT_d = nc.declare_dram_parameter("xT", [D, T], BF16, isOutput=False)
    wqkv_d = nc.declare_dram_parameter("wqkv", [D, WCOLS], BF16, isOutput=False)
    oproj_d = nc.declare_dram_parameter("oproj", [128, 4, D], BF16, isOutput=False)
    masks_d = nc.declare_dram_parameter("masks", [128, 2, 128], BF16, isOutput=False)
    ones_d = nc.declare_dram_parameter("ones", [128, 16 * 80], BF16, isOutput=False)
    out_d = nc.declare_dram_parameter("out", [D, T], BF16, isOutput=True)
    rcscr_d = nc.dram_tensor("rcscratch", [16, 1024], F32)

    with tile.TileContext(nc) as tc:
        with (
            tc.tile_pool(name="pers", bufs=1) as pers,
            tc.tile_pool(name="xt", bufs=20) as xtp,
            tc.tile_pool(name="ep", bufs=4) as epool,
            tc.tile_pool(name="work", bufs=2) as work,
            tc.tile_pool(name="psum", bufs=1, space="PSUM") as psum,
        ):
            # ---- persistent SBUF ----
            wqkv_sb = pers.tile([128, KT, WCOLS], BF16, tag="wqkv")
            oproj_sb = pers.tile([128, 4, D], BF16, tag="oproj")
            masks_sb = pers.tile([128, 2, 128], BF16, tag="masks")
            qt = [pers.tile([128, T], BF16, tag=f"qt{m}", name=f"qt{m}")
                  for m in range(4)]
            kp = [pers.tile([128, T], BF16, tag=f"kp{k}", name=f"kp{k}")
                  for k in range(2)]
            vT = pers.tile([128, T], BF16, tag="vT")
            vs = [pers.tile([128, 16 * 80], BF16, tag=f"vs{k}", name=f"vs{k}")
                  for k in range(2)]
            ctx = [pers.tile([128, T], BF16, tag=f"ctx{m}", name=f"ctx{m}")
                   for m in range(4)]

            # ---- weight/constant DMAs (w[k] just ahead of x[k]) ----
            wq_r = wqkv_d[:].rearrange("(k p) c -> p k c", p=128)
            xts = {}

            def load_x_half(half, with_w=False):
                ts = slice(TB * half, TB * half + TB)
                tiles = []
                for k in range(KT):
                    if with_w:
                        nc.sync.dma_start(wqkv_sb[:, k, :], wq_r[:, k, :])
                    xt = xtp.tile([128, TB], BF16, tag=f"xt{half}", bufs=16,
                                  name="xt")
                    nc.sync.dma_start(xt, xT_d[128 * k: 128 * k + 128, ts])
                    tiles.append(xt)
                    if with_w and k == 3:
                        nc.sync.dma_start(masks_sb, masks_d[:])
                        nc.sync.dma_start(vs[0], ones_d[:])
                        nc.sync.dma_start(vs[1], ones_d[:])
                xts[half] = tiles

            def v2(ap):
                return ap.rearrange("p (h q) -> p h q", h=2)

            # ---- phase 1: one 16-matmul chain per (m, token-half) ----
            def p1_group(mi, half, eng):
                ts = slice(TB * half, TB * half + TB)
                ps = psum.tile([128, TB], F32, tag="s2", bufs=3, name="p1ps")
                copy = eng.copy if eng is nc.scalar else eng.tensor_copy
                # matmul out must stay within one PSUM bank (512 f32 cols);
                # evictions per 512-half so the first hides behind the
                # second half's accumulation chain
                for sub in (0, 512):
                    for k in range(KT):
                        nc.tensor.matmul(
                            ps[:, sub: sub + 512],
                            wqkv_sb[:, k, 128 * mi: 128 * mi + 128],
                            xts[half][k][:, sub: sub + 512],
                            start=(k == 0),
                            stop=(k == KT - 1),
                        )
                    sl = slice(TB * half + sub, TB * half + sub + 512)
                    psl = ps[:, sub: sub + 512]
                    if mi < 4:
                        copy(qt[mi][:, sl], psl)
                    elif mi == 4:
                        copy(kp[0][0:64, sl], psl[0:64, :])
                        copy(kp[1][64:128, sl], psl[64:128, :])
                    else:
                        copy(vT[:, sl], psl)
                if mi == 4:
                    # kT duplicates at the other partition half
                    nc.sync.dma_start(kp[0][64:128, ts], kp[0][0:64, ts])
                    nc.sync.dma_start(kp[1][0:64, ts], kp[1][64:128, ts])
                if mi == 5:
                    pass
                    # v transposed into vs via one XBAR dma per kv head
                    # (3D out: key j lands at partition j%128, chunk j//128;
                    # ones at col 64 of each 80-block -> softmax denominator).
                    # Issued on the ACT queue: these are ~us-scale serial ops
                    # that would head-block every dma dispatch on Sync.
                    for kv in range(2):
                        vsr = vs[kv][:].rearrange("p (c e) -> p c e", e=80)
                        nc.sync.dma_start_transpose(
                            vsr[:, 8 * half: 8 * half + 8, 0:64],
                            vT[64 * kv: 64 * kv + 64, ts],
                        )

            # ---- phase 2: attention round per (head pair m, 512-query a) ----
            pending_norm = []

            def flush_norm(keep=0):
                while len(pending_norm) > keep:
                    pending_norm.pop(0)()

            def p2_S(m, a, defer_all=False):
                """S/exp part of an attention round. With defer_all, no ctx
                matmuls are emitted (caller interposes independent PE work
                and calls p2_fin); otherwise drains run 2 chunks behind."""
                kv = m // 2
                nj = 4 * (a + 1)
                qb = 512 * a
                state = {"m": m, "a": a, "kv": kv, "pend": []}
                pend = state["pend"]

                def drain(last):
                    pE, pjc, plo = pend.pop(0)
                    for h2 in range(2):
                        nc.tensor.matmul(
                            state["ctxAB"][:, 512 * h2 + plo: 512 * h2 + 512],
                            vs[kv][:, 80 * pjc: 80 * pjc + 65],
                            pE[:, 512 * h2 + plo: 512 * h2 + 512],
                            start=(pjc == 0),
                            stop=last,
                        )
                state["drain"] = drain

                if not defer_all:
                    state["ctxAB"] = psum.tile([65, 1024], F32, tag="ctx",
                                               bufs=1, name="ctxAB")
                for jc in range(nj):
                    o = jc - 4 * a
                    lo = (0, 128, 256, 384)[o] if o >= 0 else 0
                    if jc == 2:
                        flush_norm(1)
                    S = psum.tile([128, 1024], F32, tag="s2", bufs=3, name="S")
                    for h2 in range(2):
                        nc.tensor.matmul(
                            S[:, 512 * h2 + lo: 512 * h2 + 512],
                            kp[kv][64 * h2: 64 * h2 + 64,
                                   128 * jc: 128 * jc + 128],
                            qt[m][64 * h2: 64 * h2 + 64, qb + lo: qb + 512],
                            start=True,
                            stop=True,
                            tile_position=(64 * h2, 0),
                        )
                    if o >= 0:
                        nc.vector.tensor_add(
                            v2(S)[:, :, lo: lo + 128],
                            v2(S)[:, :, lo: lo + 128],
                            masks_sb,
                        )
                    E = epool.tile([128, 1024], BF16, tag="E", bufs=8, name="E")
                    if lo == 0:
                        nc.scalar.activation(E, S, EXP, scale=0.125)
                    else:
                        nc.scalar.activation(
                            v2(E)[:, :, lo:512], v2(S)[:, :, lo:512],
                            EXP, scale=0.125,
                        )
                    pend.append((E, jc, lo))
                    if not defer_all and len(pend) > 2:
                        drain(False)
                return state

            def p2_fin(state):
                m, a, pend = state["m"], state["a"], state["pend"]
                drain = state["drain"]
                if "ctxAB" not in state:
                    state["ctxAB"] = psum.tile([65, 1024], F32, tag="ctx",
                                               bufs=1, name="ctxAB")
                ctxAB = state["ctxAB"]
                while pend:
                    drain(len(pend) == 1)

                # denominator path starts immediately (DMA/rcp only);
                # the DVE multiplies are deferred one round so they never
                # stall this round's pipeline waiting on the DRAM bounce.
                cu = work.tile([65, 1024], F32, tag="cu", bufs=3, name="cu")
                nc.vector.tensor_copy(cu, ctxAB)
                den128 = work.tile([128, 8], F32, tag="d128", bufs=2,
                                   name="den128")
                nc.sync.dma_start(den128, cu[64:65, :])
                rcp = work.tile([128, 8], F32, tag="rcp", bufs=2, name="rcp")
                nc.vector.reciprocal(rcp, den128)
                ma = m * 4 + a
                nc.sync.dma_start(rcscr_d[ma: ma + 1, :], rcp)
                bcs = work.tile([64, 1024], F32, tag="bcs", bufs=3,
                                name="bcs")
                nc.sync.dma_start(
                    bcs, rcscr_d[ma: ma + 1, :].partition_broadcast(64)
                )

                def _norm(cu=cu, bcs=bcs, m=m, a=a):
                    isl = slice(512 * a, 512 * a + 512)
                    nc.vector.tensor_mul(
                        ctx[m][0:64, isl], cu[0:64, 0:512], bcs[:, 0:512]
                    )
                    tmpB = work.tile([64, 512], BF16, tag="tb", bufs=2,
                                     name="tmpB")
                    nc.vector.tensor_mul(
                        tmpB, cu[0:64, 512:1024], bcs[:, 512:1024]
                    )
                    nc.sync.dma_start(ctx[m][64:128, isl], tmpB)

                pending_norm.append(_norm)

            def p2_round(m, a):
                p2_fin(p2_S(m, a))

            # ---- phase 3: out.T group per (128-outcol chunk, token half) ----
            def p3_group(rc, th, eng=None):
                eng = eng or nc.vector
                tsl = slice(TB * th, TB * th + TB)
                ps3 = psum.tile([128, TB], F32, tag="s2", bufs=3, name="p3ps")
                for sub in (0, 512):
                    for m in range(4):
                        nc.tensor.matmul(
                            ps3[:, sub: sub + 512],
                            oproj_sb[:, m, 128 * rc: 128 * rc + 128],
                            ctx[m][:, TB * th + sub: TB * th + sub + 512],
                            start=(m == 0),
                            stop=(m == 3),
                        )
                ostage = work.tile([128, TB], BF16, tag="ostage", bufs=3,
                                   name="ostage")
                if eng is nc.scalar:
                    eng.copy(ostage, ps3)
                else:
                    eng.tensor_copy(ostage, ps3)
                nc.sync.dma_start(out_d[128 * rc: 128 * rc + 128, tsl], ostage)

            def p3q_group(rc, q, eng=None):
                eng = eng or nc.vector
                qsl = slice(512 * q, 512 * q + 512)
                ps3 = psum.tile([128, TB], F32, tag="s2", bufs=3, name="p3qps")
                for m in range(4):
                    nc.tensor.matmul(
                        ps3[:, 0:512],
                        oproj_sb[:, m, 128 * rc: 128 * rc + 128],
                        ctx[m][:, qsl],
                        start=(m == 0),
                        stop=(m == 3),
                    )
                ostq = work.tile([128, 512], BF16, tag="ostq", bufs=3,
                                 name="ostq")
                if eng is nc.scalar:
                    eng.copy(ostq, ps3[:, 0:512])
                else:
                    eng.tensor_copy(ostq, ps3[:, 0:512])
                nc.sync.dma_start(out_d[128 * rc: 128 * rc + 128, qsl], ostq)

            # ================= emission =================
            # stage 1: a=0 rounds cross-pipelined with phase-1 chains (the
            # 4-chunk rounds are too short to hide their own exp latency)
            load_x_half(0, with_w=True)
            p1_group(4, 0, nc.scalar)
            load_x_half(1)
            p1_group(5, 0, nc.scalar)
            p1_group(0, 0, nc.vector)
            st = p2_S(0, 0, defer_all=True)
            p1_group(1, 0, nc.scalar)
            p2_fin(st)
            st = p2_S(1, 0, defer_all=True)
            p1_group(2, 0, nc.vector)
            p2_fin(st)
            st = p2_S(2, 0, defer_all=True)
            p1_group(3, 0, nc.scalar)
            p2_fin(st)
            st = p2_S(3, 0, defer_all=True)

            p1_group(4, 1, nc.vector)
            p2_fin(st)
            st = p2_S(0, 1, defer_all=True)
            p1_group(5, 1, nc.vector)
            nc.sync.dma_start(oproj_sb, oproj_d[:])
            p2_fin(st)
            p2_round(1, 1)
            p1_group(0, 1, nc.vector)
            p2_round(2, 1)
            p1_group(1, 1, nc.vector)
            p2_round(3, 1)
            p1_group(2, 1, nc.vector)
            p1_group(3, 1, nc.vector)

            flush_norm(0)
            p3c = 0
            for m in range(4):
                p2_round(m, 2)
                for _ in range(4):
                    p3_group(p3c, 0)
                    p3c += 1
            q2c = 0
            for m in range(4):
                p2_round(m, 3)
                if m == 0:
                    flush_norm(1)  # last a=2 norm -> quarter-2 ctx complete
                else:
                    for _ in range(2):
                        p3q_group(q2c, 2)
                        q2c += 1
            while q2c < 16:
                p3q_group(q2c, 2, nc.scalar if q2c % 2 == 0 else nc.vector)
                q2c += 1
            flush_norm(0)
            for rc in range(16):
                p3q_group(rc, 3, nc.scalar if rc % 2 == 0 else nc.vector)

    nc.compile()
    return nc


def _host_inputs(x, q_proj, k_proj, v_proj, o_proj):
    """Per-core input dicts (numpy, bf16)."""
    import ml_dtypes
    bf = ml_dtypes.bfloat16

    jj = np.arange(128)[:, None]
    cc = np.arange(128)[None, :]
    tri = np.where(jj <= cc, 0.0, NEG).astype(np.float32)
    masks = np.stack([tri, tri], axis=1).astype(bf)  # [128, 2, 128]
    ones = np.ones((128, 16 * 80), dtype=np.float32).astype(bf)

    xT = [np.ascontiguousarray(x[b].T).astype(bf) for b in range(B)]
    in_maps = []
    for c in range(N_CORES):
        b, g = divmod(c, GROUPS)
        wqkv = np.concatenate(
            [
                q_proj[QCOLS * g: QCOLS * g + QCOLS].T,
                k_proj[KCOLS * g: KCOLS * g + KCOLS].T,
                v_proj[KCOLS * g: KCOLS * g + KCOLS].T,
            ],
            axis=1,
        ).astype(bf)
        op = o_proj[:, QCOLS * g: QCOLS * g + QCOLS].T  # [512 e, 2048 r]
        op = np.ascontiguousarray(
            op.reshape(4, 128, D).transpose(1, 0, 2)
        ).astype(bf)
        in_maps.append(
            {
                "xT": xT[b],
                "wqkv": np.ascontiguousarray(wqkv),
                "oproj": op,
                "masks": masks,
                "ones": ones,
            }
        )
    return in_maps


def run(x, q_proj, k_proj, v_proj, o_proj, trace=False):
    """Run on hardware; returns (output [B,T,D] f32, BassKernelResults)."""
    if "nc" not in _cache:
        _cache["nc"] = _build()
    nc = _cache["nc"]
    in_maps = _host_inputs(x, q_proj, k_proj, v_proj, o_proj)
    res = run_bass_kernel_spmd(
        nc, in_maps, core_ids=list(range(N_CORES)), trace=trace
    )
    parts = [res.results[c]["out"] for c in range(N_CORES)]
    out = np.empty((B, T, D), dtype=np.float32)
    for b in range(B):
        acc = parts[4 * b].astype(np.float64)
        for g in range(1, GROUPS):
            acc += parts[4 * b + g].astype(np.float64)
        out[b] = acc.T.astype(np.float32)
    return out, res


def kernel(x, q_proj, k_proj, v_proj, o_proj, hq=None, hk=None, **_unused):
    x = np.asarray(x, dtype=np.float32)
    q_proj = np.asarray(q_proj, dtype=np.float32)
    k_proj = np.asarray(k_proj, dtype=np.float32)
    v_proj = np.asarray(v_proj, dtype=np.float32)
    o_proj = np.asarray(o_proj, dtype=np.float32)
    assert x.shape == (B, T, D), x.shape
    trace = bool(os.environ.get("KERNEL_TRACE"))
    out, _ = run(x, q_proj, k_proj, v_proj, o_proj, trace=trace)
    return out
